# revision 5
# baseline (speedup 1.0000x reference)
"""Trainium2 Bass kernel for nn_DecoderAttentionRNN (single-step attention-GRU decoder).

Strategy (8 NeuronCores, no collectives — 3 SPMD launches with host-side
gather/re-feed of tiny partial vectors between them; all math on device):

  K1: encoder_outputs sharded over S (512 rows/core); each core computes its
      slice of Anxn = Va(tanh(Wa h + Ua enc)) and reduces it to per-channel
      sum(exp(A)) plus (core 0) the A[s=0] column.  The three vocab-wide GRU
      input matmuls Ux (U: [256,10000]) are sharded over the vocab
      (contraction) dim: 1250 columns/core -> partial [256] rows per gate.
  K2: per-core partials from K1 are summed ON DEVICE (ones-vector matmuls,
      contraction over the 8-core axis), then each core computes the full GRU
      (attention context Ci, gates z/r/c, new_hidden) redundantly and its
      1250-wide shard of the V projection (V_w sharded over vocab rows),
      emitting raw logits + local (max, sumexp).
  K3: global log-softmax normalization: each core combines the 8 local
      (max, sumexp) stats on device and subtracts the global logsumexp from
      its logits shard.

Host work between launches is only slicing/stacking/concatenation.
"""

import numpy as np

VOCAB = 10000
H = 256
S = 4096
NC = 8
SC = S // NC            # 512 encoder rows per core
VS = VOCAB // NC        # 1250 vocab entries per core
VSP = 1280              # padded to 10 chunks of 128
NV = VSP // 128         # 10
F32 = None              # set after imports

_KERNELS = None
LAST_EXEC_NS = []
LAST_RESULTS = {}       # filled when kernel(..., trace=True)


def _imports():
    import concourse.bass as bass
    import concourse.bacc as bacc
    import concourse.mybir as mybir
    import concourse.tile as tile
    from concourse.bass_utils import run_bass_kernel_spmd
    return bass, bacc, mybir, tile, run_bass_kernel_spmd


def _build_k1():
    bass, bacc, mybir, tile, _ = _imports()
    f32 = mybir.dt.float32
    nc = bacc.Bacc("TRN2", target_bir_lowering=False, debug=False, num_devices=NC)

    enc_in = nc.dram_tensor("enc_in", [128, 2, SC], f32, kind="ExternalInput")
    ua_in = nc.dram_tensor("ua_in", [128, 2, 2, 128], f32, kind="ExternalInput")
    va_in = nc.dram_tensor("va_in", [128, 2, 2, 128], f32, kind="ExternalInput")
    wa_in = nc.dram_tensor("wa_in", [128, 2, 2, 128], f32, kind="ExternalInput")
    hp_in = nc.dram_tensor("hp_in", [128, 2], f32, kind="ExternalInput")
    ab_in = nc.dram_tensor("ab_in", [128, 2], f32, kind="ExternalInput")
    vab_in = nc.dram_tensor("vab_in", [128, 2], f32, kind="ExternalInput")
    x_in = nc.dram_tensor("x_in", [128, NV], f32, kind="ExternalInput")
    ut_ins = [nc.dram_tensor(f"u{g}t_in", [128, NV, H], f32, kind="ExternalInput")
              for g in "zrh"]

    se_o = nc.dram_tensor("se_o", [128, 2], f32, kind="ExternalOutput")
    a0_o = nc.dram_tensor("a0_o", [128, 2], f32, kind="ExternalOutput")
    ux_o = nc.dram_tensor("ux_o", [1, 3 * H], f32, kind="ExternalOutput")

    with tile.TileContext(nc) as tc:
        with (
            tc.tile_pool(name="sb", bufs=1) as sb,
            tc.tile_pool(name="ps", bufs=1, space="PSUM") as ps,
        ):
            enc_sb = sb.tile([128, 2, SC], f32, name="enc")
            ua_sb = sb.tile([128, 2, 2, 128], f32, name="ua")
            va_sb = sb.tile([128, 2, 2, 128], f32, name="va")
            wa_sb = sb.tile([128, 2, 2, 128], f32, name="wa")
            hp_sb = sb.tile([128, 2], f32, name="hp")
            ab_sb = sb.tile([128, 2], f32, name="ab")
            vab_sb = sb.tile([128, 2], f32, name="vab")
            x_sb = sb.tile([128, NV], f32, name="x")
            ut_sbs = [sb.tile([128, NV, H], f32, name=f"ut{g}") for g in "zrh"]

            nc.sync.dma_start(ua_sb[:], ua_in[:])
            nc.sync.dma_start(wa_sb[:], wa_in[:])
            nc.sync.dma_start(hp_sb[:], hp_in[:])
            nc.sync.dma_start(ab_sb[:], ab_in[:])
            nc.sync.dma_start(vab_sb[:], vab_in[:])
            nc.sync.dma_start(enc_sb[:], enc_in[:])
            nc.sync.dma_start(va_sb[:], va_in[:])
            nc.sync.dma_start(x_sb[:], x_in[:])
            for t, d in zip(ut_sbs, ut_ins):
                nc.sync.dma_start(t[:], d[:])

            # attention bias per channel k: Wa_w @ h + (Wa_b + Ua_b)
            p_wah = ps.tile([128, 2], f32, name="p_wah")
            for kc in range(2):
                for jc in range(2):
                    nc.tensor.matmul(p_wah[:, kc:kc + 1], wa_sb[:, jc, kc, :],
                                     hp_sb[:, jc:jc + 1],
                                     start=(jc == 0), stop=(jc == 1))
            attb_sb = sb.tile([128, 2], f32, name="attb")
            nc.vector.tensor_tensor(attb_sb[:], p_wah[:], ab_sb[:],
                                    op=mybir.AluOpType.add)

            # U[k, s] = Ua_w @ encT ; T = tanh(U + attb)
            t_sb = sb.tile([128, 2, SC], f32, name="t")
            p_us = [ps.tile([128, SC], f32, name=f"p_u{kc}") for kc in range(2)]
            for kc in range(2):
                for jc in range(2):
                    nc.tensor.matmul(p_us[kc][:], ua_sb[:, jc, kc, :],
                                     enc_sb[:, jc, :],
                                     start=(jc == 0), stop=(jc == 1))
                nc.scalar.activation(t_sb[:, kc, :], p_us[kc][:],
                                     mybir.ActivationFunctionType.Tanh,
                                     bias=attb_sb[:, kc:kc + 1])

            # A[h, s] = Va_w @ T  (+ Va_b folded into the exp / a0 below)
            p_as = [ps.tile([128, SC], f32, name=f"p_a{hc}") for hc in range(2)]
            for hc in range(2):
                for kc in range(2):
                    nc.tensor.matmul(p_as[hc][:], va_sb[:, kc, hc, :],
                                     t_sb[:, kc, :],
                                     start=(kc == 0), stop=(kc == 1))

            escr = sb.tile([128, SC], f32, name="escr")
            se_sb = sb.tile([128, 2], f32, name="se")
            a0_sb = sb.tile([128, 2], f32, name="a0")
            for hc in range(2):
                nc.scalar.activation(escr[:], p_as[hc][:],
                                     mybir.ActivationFunctionType.Exp,
                                     bias=vab_sb[:, hc:hc + 1],
                                     accum_out=se_sb[:, hc:hc + 1])
                nc.vector.tensor_tensor(a0_sb[:, hc:hc + 1], p_as[hc][:, 0:1],
                                        vab_sb[:, hc:hc + 1],
                                        op=mybir.AluOpType.add)
            nc.sync.dma_start(se_o[:], se_sb[:])
            nc.sync.dma_start(a0_o[:], a0_sb[:])

            # Ux partials over this core's vocab slice: [1, 256] per gate
            ux_sb = sb.tile([1, 3 * H], f32, name="uxrow")
            for g in range(3):
                p_ux = ps.tile([1, H], f32, name=f"p_ux{g}")
                for i in range(NV):
                    nc.tensor.matmul(p_ux[:], x_sb[:, i:i + 1],
                                     ut_sbs[g][:, i, :],
                                     start=(i == 0), stop=(i == NV - 1))
                nc.vector.tensor_copy(ux_sb[:, g * H:(g + 1) * H], p_ux[:])
            nc.sync.dma_start(ux_o[:], ux_sb[:])

    nc.compile()
    return nc


def _build_k2():
    bass, bacc, mybir, tile, _ = _imports()
    f32 = mybir.dt.float32
    AT = mybir.ActivationFunctionType
    OP = mybir.AluOpType
    nc = bacc.Bacc("TRN2", target_bir_lowering=False, debug=False, num_devices=NC)

    pzc_in = nc.dram_tensor("pzc_in", [NC, 2 * H], f32, kind="ExternalInput")
    pr_in = nc.dram_tensor("pr_in", [NC, H], f32, kind="ExternalInput")
    seT_in = nc.dram_tensor("seT_in", [NC, H], f32, kind="ExternalInput")
    a0_in = nc.dram_tensor("a0_in", [128, 2], f32, kind="ExternalInput")
    enc0_in = nc.dram_tensor("enc0_in", [128, 2], f32, kind="ExternalInput")
    hp_in = nc.dram_tensor("hp_in", [128, 2], f32, kind="ExternalInput")
    hf_in = nc.dram_tensor("hf_in", [1, H], f32, kind="ExternalInput")
    negbr_in = nc.dram_tensor("negbr_in", [128, 2], f32, kind="ExternalInput")
    bz_in = nc.dram_tensor("bz_in", [1, H], f32, kind="ExternalInput")
    bc_in = nc.dram_tensor("bc_in", [1, H], f32, kind="ExternalInput")
    czt_in = nc.dram_tensor("czt_in", [128, 2, H], f32, kind="ExternalInput")
    cht_in = nc.dram_tensor("cht_in", [128, 2, H], f32, kind="ExternalInput")
    wzt_in = nc.dram_tensor("wzt_in", [128, 2, H], f32, kind="ExternalInput")
    wht_in = nc.dram_tensor("wht_in", [128, 2, H], f32, kind="ExternalInput")
    crt_in = nc.dram_tensor("crt_in", [128, 2, 2, 128], f32, kind="ExternalInput")
    wrt_in = nc.dram_tensor("wrt_in", [128, 2, 2, 128], f32, kind="ExternalInput")
    vt_in = nc.dram_tensor("vt_in", [128, 2, VSP], f32, kind="ExternalInput")
    vb_in = nc.dram_tensor("vb_in", [1, VSP], f32, kind="ExternalInput")

    logits_o = nc.dram_tensor("logits_o", [1, VSP], f32, kind="ExternalOutput")
    vstat_o = nc.dram_tensor("vstat_o", [1, 2], f32, kind="ExternalOutput")
    nh_o = nc.dram_tensor("nh_o", [1, H], f32, kind="ExternalOutput")

    NSL = [(0, 512), (512, 512), (1024, VSP - 1024)]  # V out column slices

    with tile.TileContext(nc) as tc:
        with (
            tc.tile_pool(name="sb", bufs=1) as sb,
            tc.tile_pool(name="ps", bufs=1, space="PSUM") as ps,
        ):
            tiles = {}
            for name, dram, shape in [
                ("pzc", pzc_in, [NC, 2 * H]), ("pr", pr_in, [NC, H]),
                ("seT", seT_in, [NC, H]), ("a0", a0_in, [128, 2]),
                ("enc0", enc0_in, [128, 2]), ("hp", hp_in, [128, 2]),
                ("hf", hf_in, [1, H]), ("negbr", negbr_in, [128, 2]),
                ("bz", bz_in, [1, H]), ("bc", bc_in, [1, H]),
                ("czt", czt_in, [128, 2, H]), ("cht", cht_in, [128, 2, H]),
                ("wzt", wzt_in, [128, 2, H]), ("wht", wht_in, [128, 2, H]),
                ("crt", crt_in, [128, 2, 2, 128]), ("wrt", wrt_in, [128, 2, 2, 128]),
                ("vt", vt_in, [128, 2, VSP]), ("vb", vb_in, [1, VSP]),
            ]:
                t = sb.tile(shape, f32, name=name)
                nc.sync.dma_start(t[:], dram[:])
                tiles[name] = t

            ones8 = sb.tile([NC, 1], f32, name="ones8")
            nc.vector.memset(ones8[:], 1.0)
            ones1 = sb.tile([1, 1], f32, name="ones1")
            nc.vector.memset(ones1[:], 1.0)

            # --- cross-core sums (contraction over the 8-core axis) ---
            p_se = ps.tile([128, 2], f32, name="p_se")
            for hc in range(2):
                nc.tensor.matmul(p_se[:, hc:hc + 1],
                                 tiles["seT"][:, hc * 128:(hc + 1) * 128],
                                 ones8[:], start=True, stop=True)

            # lse per channel (partition layout), then Ci = (A0 - lse) * enc0
            lse_sb = sb.tile([128, 2], f32, name="lse")
            nc.scalar.activation(lse_sb[:], p_se[:], AT.Ln)
            ci_sb = sb.tile([128, 2], f32, name="ci")
            nc.vector.tensor_tensor(ci_sb[:], tiles["a0"][:], lse_sb[:],
                                    op=OP.subtract)
            nc.vector.tensor_tensor(ci_sb[:], ci_sb[:], tiles["enc0"][:],
                                    op=OP.mult)

            # --- r gate (partition layout [128, 2]) ---
            p_r = ps.tile([128, 2], f32, name="p_r")
            for mc in range(2):
                nc.tensor.matmul(p_r[:, mc:mc + 1],
                                 tiles["pr"][:, mc * 128:(mc + 1) * 128],
                                 ones8[:], start=True, stop=False,
                                 skip_group_check=True)
                for jc in range(2):
                    nc.tensor.matmul(p_r[:, mc:mc + 1],
                                     tiles["wrt"][:, jc, mc, :],
                                     tiles["hp"][:, jc:jc + 1],
                                     start=False, stop=False,
                                     skip_group_check=True)
                for jc in range(2):
                    nc.tensor.matmul(p_r[:, mc:mc + 1],
                                     tiles["crt"][:, jc, mc, :],
                                     ci_sb[:, jc:jc + 1],
                                     start=False, stop=(jc == 1),
                                     skip_group_check=True)
            er_sb = sb.tile([128, 2], f32, name="er")
            for mc in range(2):
                nc.scalar.activation(er_sb[:, mc:mc + 1], p_r[:, mc:mc + 1],
                                     AT.Exp, scale=-1.0,
                                     bias=tiles["negbr"][:, mc:mc + 1])
            r_sb = sb.tile([128, 2], f32, name="r")
            nc.vector.tensor_scalar_add(er_sb[:], er_sb[:], 1.0)
            nc.vector.reciprocal(r_sb[:], er_sb[:])
            rh_sb = sb.tile([128, 2], f32, name="rh")
            nc.vector.tensor_tensor(rh_sb[:], r_sb[:], tiles["hp"][:], op=OP.mult)

            # --- z and c gate pre-activations (free layout [1, 256]) ---
            p_z = ps.tile([1, H], f32, name="p_z")
            p_c = ps.tile([1, H], f32, name="p_c")
            nc.tensor.matmul(p_z[:], ones8[:], tiles["pzc"][:, 0:H],
                             start=True, stop=False, skip_group_check=True)
            nc.tensor.matmul(p_z[:], ones1[:], tiles["bz"][:],
                             start=False, stop=False, skip_group_check=True)
            nc.tensor.matmul(p_c[:], ones8[:], tiles["pzc"][:, H:2 * H],
                             start=True, stop=False, skip_group_check=True)
            nc.tensor.matmul(p_c[:], ones1[:], tiles["bc"][:],
                             start=False, stop=False, skip_group_check=True)
            for jc in range(2):
                nc.tensor.matmul(p_z[:], tiles["hp"][:, jc:jc + 1],
                                 tiles["wzt"][:, jc, :],
                                 start=False, stop=False, skip_group_check=True)
            for jc in range(2):
                nc.tensor.matmul(p_z[:], ci_sb[:, jc:jc + 1],
                                 tiles["czt"][:, jc, :],
                                 start=False, stop=(jc == 1),
                                 skip_group_check=True)
            for jc in range(2):
                nc.tensor.matmul(p_c[:], ci_sb[:, jc:jc + 1],
                                 tiles["cht"][:, jc, :],
                                 start=False, stop=False, skip_group_check=True)
            for jc in range(2):
                nc.tensor.matmul(p_c[:], rh_sb[:, jc:jc + 1],
                                 tiles["wht"][:, jc, :],
                                 start=False, stop=(jc == 1),
                                 skip_group_check=True)

            # z = sigmoid(p_z) via 1/(1+exp(-x)); c = tanh(p_c) via 2*sigm(2x)-1
            z_sb = sb.tile([1, H], f32, name="z")
            nc.scalar.activation(z_sb[:], p_z[:], AT.Exp, scale=-1.0)
            nc.vector.tensor_scalar_add(z_sb[:], z_sb[:], 1.0)
            nc.vector.reciprocal(z_sb[:], z_sb[:])
            c_sb = sb.tile([1, H], f32, name="c")
            nc.scalar.activation(c_sb[:], p_c[:], AT.Exp, scale=-2.0)
            nc.vector.tensor_scalar_add(c_sb[:], c_sb[:], 1.0)
            nc.vector.reciprocal(c_sb[:], c_sb[:])
            nc.vector.tensor_scalar(c_sb[:], c_sb[:], 2.0, -1.0,
                                    op0=OP.mult, op1=OP.add)

            # nh = c + z * (h - c)
            nh_sb = sb.tile([1, H], f32, name="nh")
            d_sb = sb.tile([1, H], f32, name="d")
            nc.vector.tensor_tensor(d_sb[:], tiles["hf"][:], c_sb[:],
                                    op=OP.subtract)
            nc.vector.tensor_tensor(d_sb[:], z_sb[:], d_sb[:], op=OP.mult)
            nc.vector.tensor_tensor(nh_sb[:], c_sb[:], d_sb[:], op=OP.add)
            nc.sync.dma_start(nh_o[:], nh_sb[:])

            # transpose nh to partition layout via K=1 matmuls
            p_nh = ps.tile([128, 2], f32, name="p_nh")
            for jc in range(2):
                nc.tensor.matmul(p_nh[:, jc:jc + 1],
                                 nh_sb[0:1, jc * 128:(jc + 1) * 128],
                                 ones1[:], start=True, stop=True,
                                 skip_group_check=True)
            nhp_sb = sb.tile([128, 2], f32, name="nhp")
            nc.vector.tensor_copy(nhp_sb[:], p_nh[:])

            # V projection shard: logits[n] = V_w[n, :] @ nh + V_b[n]
            logits_sb = sb.tile([1, VSP], f32, name="logits")
            for si, (off, nsz) in enumerate(NSL):
                p_v = ps.tile([1, nsz], f32, name=f"p_v{si}")
                for jc in range(2):
                    nc.tensor.matmul(p_v[:], nhp_sb[:, jc:jc + 1],
                                     tiles["vt"][:, jc, off:off + nsz],
                                     start=(jc == 0), stop=False,
                                     skip_group_check=True)
                nc.tensor.matmul(p_v[:], ones1[:], tiles["vb"][:, off:off + nsz],
                                 start=False, stop=True, skip_group_check=True)
                nc.vector.tensor_copy(logits_sb[:, off:off + nsz], p_v[:])

            # local stats over the real 1250 entries
            vstat_sb = sb.tile([1, 2], f32, name="vstat")
            negmax_sb = sb.tile([1, 1], f32, name="negmax")
            nc.vector.tensor_reduce(negmax_sb[:], logits_sb[0:1, 0:VS],
                                    axis=mybir.AxisListType.X,
                                    op=OP.max, negate=True)
            nc.vector.tensor_scalar_mul(vstat_sb[:, 0:1], negmax_sb[:], -1.0)
            escr = sb.tile([1, VS], f32, name="escr")
            nc.scalar.activation(escr[:], logits_sb[0:1, 0:VS], AT.Exp,
                                 bias=negmax_sb[:],
                                 accum_out=vstat_sb[:, 1:2])
            nc.sync.dma_start(logits_o[:], logits_sb[:])
            nc.sync.dma_start(vstat_o[:], vstat_sb[:])

    nc.compile()
    return nc


def _build_k3():
    bass, bacc, mybir, tile, _ = _imports()
    f32 = mybir.dt.float32
    AT = mybir.ActivationFunctionType
    OP = mybir.AluOpType
    nc = bacc.Bacc("TRN2", target_bir_lowering=False, debug=False, num_devices=NC)

    logits_in = nc.dram_tensor("logits_in", [1, VSP], f32, kind="ExternalInput")
    maxs_in = nc.dram_tensor("maxs_in", [1, NC], f32, kind="ExternalInput")
    ses_in = nc.dram_tensor("ses_in", [1, NC], f32, kind="ExternalInput")
    outp_o = nc.dram_tensor("outp_o", [1, VS], f32, kind="ExternalOutput")

    with tile.TileContext(nc) as tc:
        with tc.tile_pool(name="sb", bufs=1) as sb:
            lg = sb.tile([1, VSP], f32, name="lg")
            mx = sb.tile([1, NC], f32, name="mx")
            se = sb.tile([1, NC], f32, name="se")
            nc.sync.dma_start(lg[:], logits_in[:])
            nc.sync.dma_start(mx[:], maxs_in[:])
            nc.sync.dma_start(se[:], ses_in[:])

            negM = sb.tile([1, 1], f32, name="negM")
            nc.vector.tensor_reduce(negM[:], mx[:], axis=mybir.AxisListType.X,
                                    op=OP.max, negate=True)
            em = sb.tile([1, NC], f32, name="em")
            nc.scalar.activation(em[:], mx[:], AT.Exp, bias=negM[:])
            nc.vector.tensor_tensor(em[:], em[:], se[:], op=OP.mult)
            ssum = sb.tile([1, 1], f32, name="ssum")
            nc.vector.tensor_reduce(ssum[:], em[:], axis=mybir.AxisListType.X,
                                    op=OP.add)
            lnl = sb.tile([1, 1], f32, name="lnl")
            nc.scalar.activation(lnl[:], ssum[:], AT.Ln)
            negl = sb.tile([1, 1], f32, name="negl")
            nc.vector.tensor_tensor(negl[:], negM[:], lnl[:], op=OP.subtract)

            outp_sb = sb.tile([1, VS], f32, name="outp")
            nc.scalar.activation(outp_sb[:], lg[0:1, 0:VS], AT.Identity,
                                 bias=negl[:])
            nc.sync.dma_start(outp_o[:], outp_sb[:])

    nc.compile()
    return nc


def _get_kernels():
    global _KERNELS
    if _KERNELS is None:
        _KERNELS = (_build_k1(), _build_k2(), _build_k3())
    return _KERNELS


def _part2(v):
    """[256] vector -> [128, 2] partition-major chunks."""
    return np.ascontiguousarray(v.reshape(2, 128).T)


def kernel(input_ids, hidden, encoder_outputs, params, trace=False):
    _, _, _, _, run_bass_kernel_spmd = _imports()
    k1, k2, k3 = _get_kernels()
    cores = list(range(NC))
    LAST_EXEC_NS.clear()

    def run(nc, in_maps):
        res = run_bass_kernel_spmd(nc, in_maps, cores, trace=trace)
        if trace:
            LAST_EXEC_NS.append(res.exec_time_ns)
        return res.results

    p = {k: np.asarray(v, dtype=np.float32) for k, v in params.items()}
    input_ids = np.asarray(input_ids)
    hidden = np.asarray(hidden, dtype=np.float32)
    enc = np.asarray(encoder_outputs, dtype=np.float32)

    h = hidden.reshape(H)
    hp = _part2(h)
    hf = np.ascontiguousarray(h.reshape(1, H))

    # ---- K1 input prep ----
    def chunks2(mt):  # M_w.T [256,256] -> [128, jc, kc, 128]
        return np.ascontiguousarray(
            mt.reshape(2, 128, 2, 128).transpose(1, 0, 2, 3))

    ua_a = chunks2(p["Ua_w"].T)
    va_a = chunks2(p["Va_w"].T)
    wa_a = chunks2(p["Wa_w"].T)
    ab_a = _part2(p["Wa_b"] + p["Ua_b"])
    vab_a = _part2(p["Va_b"])

    x = p["emb"][int(input_ids[0])]
    x_pad = np.zeros(NC * VSP, np.float32)
    x_pad[:VOCAB] = x

    ut_pads = {}
    for g, name in zip("zrh", ["Uz_w", "Ur_w", "Uh_w"]):
        tp = np.zeros((NC * VSP, H), np.float32)
        tp[:VOCAB] = p[name].T
        ut_pads[g] = tp

    in1 = []
    for c in range(NC):
        enc_c = enc[c * SC:(c + 1) * SC, 0, :]          # [512, 256]
        enc_a = np.ascontiguousarray(
            enc_c.T.reshape(2, 128, SC).transpose(1, 0, 2))
        m = {
            "enc_in": enc_a, "ua_in": ua_a, "va_in": va_a, "wa_in": wa_a,
            "hp_in": hp, "ab_in": ab_a, "vab_in": vab_a,
            "x_in": np.ascontiguousarray(
                x_pad[c * VSP:(c + 1) * VSP].reshape(NV, 128).T),
        }
        for g, name in zip("zrh", ["uzt_in", "urt_in", "uht_in"]):
            m[name] = np.ascontiguousarray(
                ut_pads[g][c * VSP:(c + 1) * VSP].reshape(NV, 128, H)
                .transpose(1, 0, 2))
        in1.append(m)
    r1 = run(k1, in1)

    # ---- K2 input prep (host: stack/slice only) ----
    ux = np.stack([r1[c]["ux_o"][0] for c in range(NC)])       # [8, 768]
    pzc = np.ascontiguousarray(
        np.concatenate([ux[:, 0:H], ux[:, 2 * H:3 * H]], axis=1))  # {Uzx, Uhx}
    pr = np.ascontiguousarray(ux[:, H:2 * H])
    seT = np.stack([r1[c]["se_o"].T.reshape(H) for c in range(NC)])
    a0 = r1[0]["a0_o"]

    vt_pad = np.zeros((H, NC, VSP), np.float32)
    vt_pad[:, :, :VS] = p["V_w"].T.reshape(H, NC, VS)
    vb_pad = np.zeros((NC, VSP), np.float32)
    vb_pad[:, :VS] = p["V_b"].reshape(NC, VS)

    def hchunks(mt):  # M_w.T [256,256] -> [128, 2, 256]
        return np.ascontiguousarray(mt.reshape(2, 128, H).transpose(1, 0, 2))

    common2 = {
        "pzc_in": pzc, "pr_in": pr, "seT_in": seT, "a0_in": a0,
        "enc0_in": _part2(enc[0, 0, :]), "hp_in": hp, "hf_in": hf,
        "negbr_in": _part2(-(p["Ur_b"] + p["Wr_b"] + p["Cr_b"])),
        "bz_in": (p["Uz_b"] + p["Wz_b"] + p["Cz_b"]).reshape(1, H),
        "bc_in": (p["Uh_b"] + p["Wh_b"] + p["Ch_b"]).reshape(1, H),
        "czt_in": hchunks(p["Cz_w"].T), "cht_in": hchunks(p["Ch_w"].T),
        "wzt_in": hchunks(p["Wz_w"].T), "wht_in": hchunks(p["Wh_w"].T),
        "crt_in": chunks2(p["Cr_w"].T), "wrt_in": chunks2(p["Wr_w"].T),
    }
    in2 = []
    for c in range(NC):
        m = dict(common2)
        m["vt_in"] = np.ascontiguousarray(
            vt_pad[:, c, :].reshape(2, 128, VSP).transpose(1, 0, 2))
        m["vb_in"] = vb_pad[c].reshape(1, VSP)
        in2.append(m)
    r2 = run(k2, in2)

    # ---- K3: global log-softmax normalization ----
    maxs = np.array([[r2[c]["vstat_o"][0, 0] for c in range(NC)]], np.float32)
    ses = np.array([[r2[c]["vstat_o"][0, 1] for c in range(NC)]], np.float32)
    in3 = [{"logits_in": r2[c]["logits_o"], "maxs_in": maxs, "ses_in": ses}
           for c in range(NC)]
    r3 = run(k3, in3)

    LAST_RESULTS.update(r1=r1, r2=r2, r3=r3)
    output = np.concatenate([r3[c]["outp_o"][0] for c in range(NC)])
    output = output.reshape(1, VOCAB)
    new_hidden = r2[0]["nh_o"].reshape(1, 1, H)
    return output, new_hidden


# revision 9
# speedup vs baseline: 1.7157x; 1.7157x over previous
"""Trainium2 Bass kernel for nn_DecoderAttentionRNN (single-step attention-GRU decoder).

Strategy (8 NeuronCores, no collectives — 3 SPMD launches with host-side
gather/re-feed of tiny partial vectors between them; all math on device):

  K1: encoder_outputs sharded over S (512 rows/core); each core computes its
      slice of Anxn = Va(tanh(Wa h + Ua enc)) and reduces it to per-channel
      sum(exp(A)) plus (core 0) the A[s=0] column.  The three vocab-wide GRU
      input matmuls Ux (U: [256,10000]) are sharded over the vocab
      (contraction) dim: 1250 columns/core -> partial [256] rows per gate.
  K2: per-core partials from K1 are summed ON DEVICE (ones-vector matmuls,
      contraction over the 8-core axis), then each core computes the full GRU
      (attention context Ci, gates z/r/c, new_hidden) redundantly and its
      1250-wide shard of the V projection (V_w sharded over vocab rows),
      emitting raw logits + local (max, sumexp).
  K3: global log-softmax normalization: each core combines the 8 local
      (max, sumexp) stats on device and subtracts the global logsumexp from
      its logits shard.

Host work between launches is only slicing/stacking/concatenation.
"""

import numpy as np

VOCAB = 10000
H = 256
S = 4096
NC = 8
SC = S // NC            # 512 encoder rows per core
VS = VOCAB // NC        # 1250 vocab entries per core
VSP = 1280              # padded to 10 chunks of 128
NV = VSP // 128         # 10
F32 = None              # set after imports

_KERNELS = None
LAST_EXEC_NS = []
LAST_RESULTS = {}       # filled when kernel(..., trace=True)


def _imports():
    import concourse.bass as bass
    import concourse.bacc as bacc
    import concourse.mybir as mybir
    import concourse.tile as tile
    from concourse.bass_utils import run_bass_kernel_spmd
    return bass, bacc, mybir, tile, run_bass_kernel_spmd


def _build_k1():
    bass, bacc, mybir, tile, _ = _imports()
    f32 = mybir.dt.float32
    nc = bacc.Bacc("TRN2", target_bir_lowering=False, debug=False, num_devices=NC)

    bf16 = mybir.dt.bfloat16
    enc_in = nc.dram_tensor("enc_in", [128, 2, SC], bf16, kind="ExternalInput")
    ua_in = nc.dram_tensor("ua_in", [128, 2, 2, 128], bf16, kind="ExternalInput")
    va_in = nc.dram_tensor("va_in", [128, 2, 2, 128], bf16, kind="ExternalInput")
    wa_in = nc.dram_tensor("wa_in", [128, 2, 2, 128], bf16, kind="ExternalInput")
    hp_in = nc.dram_tensor("hp_in", [128, 2], bf16, kind="ExternalInput")
    ab_in = nc.dram_tensor("ab_in", [128, 2], f32, kind="ExternalInput")
    vab_in = nc.dram_tensor("vab_in", [128, 2], f32, kind="ExternalInput")
    x_in = nc.dram_tensor("x_in", [128, NV], bf16, kind="ExternalInput")
    ut_ins = [nc.dram_tensor(f"u{g}t_in", [128, NV, H], bf16, kind="ExternalInput")
              for g in "zrh"]

    se_o = nc.dram_tensor("se_o", [128, 2], f32, kind="ExternalOutput")
    a0_o = nc.dram_tensor("a0_o", [128, 2], f32, kind="ExternalOutput")
    ux_o = nc.dram_tensor("ux_o", [1, 3 * H], f32, kind="ExternalOutput")

    with tile.TileContext(nc) as tc:
        with (
            tc.tile_pool(name="sb", bufs=1) as sb,
            tc.tile_pool(name="ps", bufs=1, space="PSUM") as ps,
        ):
            enc_sb = sb.tile([128, 2, SC], bf16, name="enc")
            ua_sb = sb.tile([128, 2, 2, 128], bf16, name="ua")
            va_sb = sb.tile([128, 2, 2, 128], bf16, name="va")
            wa_sb = sb.tile([128, 2, 2, 128], bf16, name="wa")
            hp_sb = sb.tile([128, 2], bf16, name="hp")
            ab_sb = sb.tile([128, 2], f32, name="ab")
            vab_sb = sb.tile([128, 2], f32, name="vab")
            x_sb = sb.tile([128, NV], bf16, name="x")
            ut_sbs = [sb.tile([128, NV, H], bf16, name=f"ut{g}") for g in "zrh"]

            nc.sync.dma_start(ua_sb[:], ua_in[:])
            nc.sync.dma_start(wa_sb[:], wa_in[:])
            nc.sync.dma_start(hp_sb[:], hp_in[:])
            nc.sync.dma_start(ab_sb[:], ab_in[:])
            nc.sync.dma_start(vab_sb[:], vab_in[:])
            nc.sync.dma_start(enc_sb[:], enc_in[:])
            nc.sync.dma_start(va_sb[:], va_in[:])
            nc.sync.dma_start(x_sb[:], x_in[:])
            for t, d in zip(ut_sbs, ut_ins):
                nc.sync.dma_start(t[:], d[:])

            # attention bias per channel k: Wa_w @ h + (Wa_b + Ua_b)
            p_wah = ps.tile([128, 2], f32, name="p_wah")
            for kc in range(2):
                for jc in range(2):
                    nc.tensor.matmul(p_wah[:, kc:kc + 1], wa_sb[:, jc, kc, :],
                                     hp_sb[:, jc:jc + 1],
                                     start=(jc == 0), stop=(jc == 1))
            attb_sb = sb.tile([128, 2], f32, name="attb")
            nc.vector.tensor_tensor(attb_sb[:], p_wah[:], ab_sb[:],
                                    op=mybir.AluOpType.add)

            # U[k, s] = Ua_w @ encT ; T = tanh(U + attb)
            t_sb = sb.tile([128, 2, SC], bf16, name="t")
            p_us = [ps.tile([128, SC], f32, name=f"p_u{kc}") for kc in range(2)]
            for kc in range(2):
                for jc in range(2):
                    nc.tensor.matmul(p_us[kc][:], ua_sb[:, jc, kc, :],
                                     enc_sb[:, jc, :],
                                     start=(jc == 0), stop=(jc == 1))
                nc.scalar.activation(t_sb[:, kc, :], p_us[kc][:],
                                     mybir.ActivationFunctionType.Tanh,
                                     bias=attb_sb[:, kc:kc + 1])

            # A[h, s] = Va_w @ T  (+ Va_b folded into the exp / a0 below)
            p_as = [ps.tile([128, SC], f32, name=f"p_a{hc}") for hc in range(2)]
            for hc in range(2):
                for kc in range(2):
                    nc.tensor.matmul(p_as[hc][:], va_sb[:, kc, hc, :],
                                     t_sb[:, kc, :],
                                     start=(kc == 0), stop=(kc == 1))

            escr = sb.tile([128, SC], f32, name="escr")
            se_sb = sb.tile([128, 2], f32, name="se")
            a0_sb = sb.tile([128, 2], f32, name="a0")
            for hc in range(2):
                nc.scalar.activation(escr[:], p_as[hc][:],
                                     mybir.ActivationFunctionType.Exp,
                                     bias=vab_sb[:, hc:hc + 1],
                                     accum_out=se_sb[:, hc:hc + 1])
                nc.vector.tensor_tensor(a0_sb[:, hc:hc + 1], p_as[hc][:, 0:1],
                                        vab_sb[:, hc:hc + 1],
                                        op=mybir.AluOpType.add)
            nc.sync.dma_start(se_o[:], se_sb[:])
            nc.sync.dma_start(a0_o[:], a0_sb[:])

            # Ux partials over this core's vocab slice: [1, 256] per gate
            ux_sb = sb.tile([1, 3 * H], f32, name="uxrow")
            for g in range(3):
                p_ux = ps.tile([1, H], f32, name=f"p_ux{g}")
                for i in range(NV):
                    nc.tensor.matmul(p_ux[:], x_sb[:, i:i + 1],
                                     ut_sbs[g][:, i, :],
                                     start=(i == 0), stop=(i == NV - 1))
                nc.vector.tensor_copy(ux_sb[:, g * H:(g + 1) * H], p_ux[:])
            nc.sync.dma_start(ux_o[:], ux_sb[:])

    nc.compile()
    return nc


def _build_k2():
    bass, bacc, mybir, tile, _ = _imports()
    f32 = mybir.dt.float32
    bf16 = mybir.dt.bfloat16
    AT = mybir.ActivationFunctionType
    OP = mybir.AluOpType
    nc = bacc.Bacc("TRN2", target_bir_lowering=False, debug=False, num_devices=NC)

    pzc_in = nc.dram_tensor("pzc_in", [NC, 2 * H], f32, kind="ExternalInput")
    pr_in = nc.dram_tensor("pr_in", [NC, H], f32, kind="ExternalInput")
    seT_in = nc.dram_tensor("seT_in", [NC, H], f32, kind="ExternalInput")
    a0_in = nc.dram_tensor("a0_in", [128, 2], f32, kind="ExternalInput")
    enc0_in = nc.dram_tensor("enc0_in", [128, 2], f32, kind="ExternalInput")
    hpb_in = nc.dram_tensor("hpb_in", [128, 2], bf16, kind="ExternalInput")
    hpf_in = nc.dram_tensor("hpf_in", [128, 2], f32, kind="ExternalInput")
    hf_in = nc.dram_tensor("hf_in", [1, H], f32, kind="ExternalInput")
    br_in = nc.dram_tensor("br_in", [128, 2], f32, kind="ExternalInput")
    bz_in = nc.dram_tensor("bz_in", [1, H], bf16, kind="ExternalInput")
    bc_in = nc.dram_tensor("bc_in", [1, H], bf16, kind="ExternalInput")
    czt_in = nc.dram_tensor("czt_in", [128, 2, H], bf16, kind="ExternalInput")
    cht_in = nc.dram_tensor("cht_in", [128, 2, H], bf16, kind="ExternalInput")
    wzt_in = nc.dram_tensor("wzt_in", [128, 2, H], bf16, kind="ExternalInput")
    wht_in = nc.dram_tensor("wht_in", [128, 2, H], bf16, kind="ExternalInput")
    crt_in = nc.dram_tensor("crt_in", [128, 2, 2, 128], bf16, kind="ExternalInput")
    wrt_in = nc.dram_tensor("wrt_in", [128, 2, 2, 128], bf16, kind="ExternalInput")
    vt_in = nc.dram_tensor("vt_in", [128, 2, NV, 128], bf16, kind="ExternalInput")
    vb_in = nc.dram_tensor("vb_in", [128, NV], f32, kind="ExternalInput")

    logits_o = nc.dram_tensor("logits_o", [128, NV], f32, kind="ExternalOutput")
    vstat_o = nc.dram_tensor("vstat_o", [1, 1], f32, kind="ExternalOutput")
    nh_o = nc.dram_tensor("nh_o", [1, H], f32, kind="ExternalOutput")

    with tile.TileContext(nc) as tc:
        with (
            tc.tile_pool(name="sb", bufs=1) as sb,
            tc.tile_pool(name="ps", bufs=1, space="PSUM") as ps,
        ):
            tiles = {}
            for name, dram, shape, dt in [
                ("seT", seT_in, [NC, H], f32), ("a0", a0_in, [128, 2], f32),
                ("enc0", enc0_in, [128, 2], f32),
                ("pzc", pzc_in, [NC, 2 * H], f32), ("pr", pr_in, [NC, H], f32),
                ("hpb", hpb_in, [128, 2], bf16), ("hpf", hpf_in, [128, 2], f32),
                ("hf", hf_in, [1, H], f32), ("br", br_in, [128, 2], f32),
                ("bz", bz_in, [1, H], bf16), ("bc", bc_in, [1, H], bf16),
                ("czt", czt_in, [128, 2, H], bf16), ("cht", cht_in, [128, 2, H], bf16),
                ("wzt", wzt_in, [128, 2, H], bf16), ("wht", wht_in, [128, 2, H], bf16),
                ("crt", crt_in, [128, 2, 2, 128], bf16),
                ("wrt", wrt_in, [128, 2, 2, 128], bf16),
                ("vt", vt_in, [128, 2, NV, 128], bf16), ("vb", vb_in, [128, NV], f32),
            ]:
                t = sb.tile(shape, dt, name=name)
                nc.sync.dma_start(t[:], dram[:])
                tiles[name] = t

            ones8f = sb.tile([NC, 1], f32, name="ones8f")
            nc.vector.memset(ones8f[:], 1.0)
            ones8 = sb.tile([NC, 1], bf16, name="ones8")
            nc.vector.memset(ones8[:], 1.0)
            ones1 = sb.tile([1, 1], bf16, name="ones1")
            nc.vector.memset(ones1[:], 1.0)
            onescol = sb.tile([128, 1], f32, name="onescol")
            nc.vector.memset(onescol[:], 1.0)

            # --- cross-core sums (contraction over the 8-core axis) ---
            p_se = ps.tile([128, 2], f32, name="p_se")
            for hc in range(2):
                nc.tensor.matmul(p_se[:, hc:hc + 1],
                                 tiles["seT"][:, hc * 128:(hc + 1) * 128],
                                 ones8f[:], start=True, stop=True)

            # lse per channel (partition layout), then Ci = (A0 - lse) * enc0
            lse_sb = sb.tile([128, 2], f32, name="lse")
            nc.scalar.activation(lse_sb[:], p_se[:], AT.Ln)
            ci_sb = sb.tile([128, 2], f32, name="ci")
            nc.vector.tensor_tensor(ci_sb[:], tiles["a0"][:], lse_sb[:],
                                    op=OP.subtract)
            nc.vector.tensor_tensor(ci_sb[:], ci_sb[:], tiles["enc0"][:],
                                    op=OP.mult)
            cib_sb = sb.tile([128, 2], bf16, name="cib")
            nc.vector.tensor_copy(cib_sb[:], ci_sb[:])
            cilo_sb = sb.tile([128, 2], bf16, name="cilo")
            nc.vector.tensor_tensor(cilo_sb[:], ci_sb[:], cib_sb[:],
                                    op=OP.subtract)

            # --- r gate (partition layout [128, 2]) ---
            p_r = ps.tile([128, 2], f32, name="p_r")
            for mc in range(2):
                nc.tensor.matmul(p_r[:, mc:mc + 1],
                                 tiles["pr"][:, mc * 128:(mc + 1) * 128],
                                 ones8f[:], start=True, stop=False,
                                 skip_group_check=True)
                for jc in range(2):
                    nc.tensor.matmul(p_r[:, mc:mc + 1],
                                     tiles["wrt"][:, jc, mc, :],
                                     tiles["hpb"][:, jc:jc + 1],
                                     start=False, stop=False,
                                     skip_group_check=True)
                for cv in (cib_sb, cilo_sb):
                    for jc in range(2):
                        nc.tensor.matmul(p_r[:, mc:mc + 1],
                                         tiles["crt"][:, jc, mc, :],
                                         cv[:, jc:jc + 1],
                                         start=False,
                                         stop=(cv is cilo_sb and jc == 1),
                                         skip_group_check=True)
            r_sb = sb.tile([128, 2], f32, name="r")
            for mc in range(2):
                nc.scalar.activation(r_sb[:, mc:mc + 1], p_r[:, mc:mc + 1],
                                     AT.Sigmoid,
                                     bias=tiles["br"][:, mc:mc + 1])
            rh_sb = sb.tile([128, 2], f32, name="rh")
            nc.vector.tensor_tensor(rh_sb[:], r_sb[:], tiles["hpf"][:], op=OP.mult)
            rhb_sb = sb.tile([128, 2], bf16, name="rhb")
            nc.vector.tensor_copy(rhb_sb[:], rh_sb[:])

            # --- z and c gate pre-activations (free layout [1, 256]) ---
            p_z = ps.tile([1, H], f32, name="p_z")
            p_c = ps.tile([1, H], f32, name="p_c")
            nc.tensor.matmul(p_z[:], ones8f[:], tiles["pzc"][:, 0:H],
                             start=True, stop=False, skip_group_check=True)
            nc.tensor.matmul(p_z[:], ones1[:], tiles["bz"][:],
                             start=False, stop=False, skip_group_check=True)
            nc.tensor.matmul(p_c[:], ones8f[:], tiles["pzc"][:, H:2 * H],
                             start=True, stop=False, skip_group_check=True)
            nc.tensor.matmul(p_c[:], ones1[:], tiles["bc"][:],
                             start=False, stop=False, skip_group_check=True)
            for jc in range(2):
                nc.tensor.matmul(p_z[:], tiles["hpb"][:, jc:jc + 1],
                                 tiles["wzt"][:, jc, :],
                                 start=False, stop=False, skip_group_check=True)
            for cv in (cib_sb, cilo_sb):
                for jc in range(2):
                    nc.tensor.matmul(p_z[:], cv[:, jc:jc + 1],
                                     tiles["czt"][:, jc, :],
                                     start=False,
                                     stop=(cv is cilo_sb and jc == 1),
                                     skip_group_check=True)
            for cv in (cib_sb, cilo_sb):
                for jc in range(2):
                    nc.tensor.matmul(p_c[:], cv[:, jc:jc + 1],
                                     tiles["cht"][:, jc, :],
                                     start=False, stop=False,
                                     skip_group_check=True)
            for jc in range(2):
                nc.tensor.matmul(p_c[:], rhb_sb[:, jc:jc + 1],
                                 tiles["wht"][:, jc, :],
                                 start=False, stop=(jc == 1),
                                 skip_group_check=True)

            z_sb = sb.tile([1, H], f32, name="z")
            nc.scalar.activation(z_sb[:], p_z[:], AT.Sigmoid)
            c_sb = sb.tile([1, H], f32, name="c")
            nc.scalar.activation(c_sb[:], p_c[:], AT.Tanh)

            # nh = c + z * (h - c)
            nh_sb = sb.tile([1, H], f32, name="nh")
            d_sb = sb.tile([1, H], f32, name="d")
            nc.vector.tensor_tensor(d_sb[:], tiles["hf"][:], c_sb[:],
                                    op=OP.subtract)
            nc.vector.tensor_tensor(d_sb[:], z_sb[:], d_sb[:], op=OP.mult)
            nc.vector.tensor_tensor(nh_sb[:], c_sb[:], d_sb[:], op=OP.add)
            nc.sync.dma_start(nh_o[:], nh_sb[:])
            nhb_sb = sb.tile([1, H], bf16, name="nhb")
            nc.vector.tensor_copy(nhb_sb[:], nh_sb[:])

            # transpose nh to partition layout via K=1 matmuls
            p_nh = ps.tile([128, 2], f32, name="p_nh")
            for jc in range(2):
                nc.tensor.matmul(p_nh[:, jc:jc + 1],
                                 nhb_sb[0:1, jc * 128:(jc + 1) * 128],
                                 ones1[:], start=True, stop=True,
                                 skip_group_check=True)
            nhp_sb = sb.tile([128, 2], bf16, name="nhp")
            nc.vector.tensor_copy(nhp_sb[:], p_nh[:])

            # V projection shard, partition layout: out[p, i] = logits[i*128+p]
            p_v = ps.tile([128, NV], f32, name="p_v")
            for i in range(NV):
                for jc in range(2):
                    nc.tensor.matmul(p_v[:, i:i + 1],
                                     tiles["vt"][:, jc, i, :],
                                     nhp_sb[:, jc:jc + 1],
                                     start=(jc == 0), stop=(jc == 1),
                                     skip_group_check=True)
            logits_sb = sb.tile([128, NV], f32, name="logits")
            nc.vector.tensor_tensor(logits_sb[:], p_v[:], tiles["vb"][:],
                                    op=OP.add)
            nc.sync.dma_start(logits_o[:], logits_sb[:])

            # local sumexp (no max subtraction; logits are O(1), pads -1e30)
            escr = sb.tile([128, NV], f32, name="escr")
            sep_sb = sb.tile([128, 1], f32, name="sep")
            nc.scalar.activation(escr[:], logits_sb[:], AT.Exp,
                                 accum_out=sep_sb[:])
            p_s = ps.tile([1, 1], f32, name="p_s")
            nc.tensor.matmul(p_s[:], sep_sb[:], onescol[:], start=True, stop=True,
                             skip_group_check=True)
            vstat_sb = sb.tile([1, 1], f32, name="vstat")
            nc.vector.tensor_copy(vstat_sb[:], p_s[:])
            nc.sync.dma_start(vstat_o[:], vstat_sb[:])

    nc.compile()
    return nc


def _build_k3():
    bass, bacc, mybir, tile, _ = _imports()
    f32 = mybir.dt.float32
    AT = mybir.ActivationFunctionType
    OP = mybir.AluOpType
    nc = bacc.Bacc("TRN2", target_bir_lowering=False, debug=False, num_devices=NC)

    logits_in = nc.dram_tensor("logits_in", [128, NV], f32, kind="ExternalInput")
    ses_in = nc.dram_tensor("ses_in", [1, NC], f32, kind="ExternalInput")
    outp_o = nc.dram_tensor("outp_o", [128, NV], f32, kind="ExternalOutput")

    with tile.TileContext(nc) as tc:
        with (
            tc.tile_pool(name="sb", bufs=1) as sb,
            tc.tile_pool(name="ps", bufs=1, space="PSUM") as ps,
        ):
            lg = sb.tile([128, NV], f32, name="lg")
            se = sb.tile([1, NC], f32, name="se")
            nc.sync.dma_start(lg[:], logits_in[:])
            nc.sync.dma_start(se[:], ses_in[:])

            ssum = sb.tile([1, 1], f32, name="ssum")
            nc.vector.tensor_reduce(ssum[:], se[:], axis=mybir.AxisListType.X,
                                    op=OP.add)
            rec = sb.tile([1, 1], f32, name="rec")
            nc.vector.reciprocal(rec[:], ssum[:])
            negl = sb.tile([1, 1], f32, name="negl")
            nc.scalar.activation(negl[:], rec[:], AT.Ln)

            # broadcast -lse to all 128 partitions via a K=1 matmul
            onesr = sb.tile([1, 128], f32, name="onesr")
            nc.vector.memset(onesr[:], 1.0)
            p_b = ps.tile([128, 1], f32, name="p_b")
            nc.tensor.matmul(p_b[:], onesr[:], negl[:], start=True, stop=True)
            neglp = sb.tile([128, 1], f32, name="neglp")
            nc.vector.tensor_copy(neglp[:], p_b[:])

            outp_sb = sb.tile([128, NV], f32, name="outp")
            nc.scalar.activation(outp_sb[:], lg[:], AT.Identity, bias=neglp[:])
            nc.sync.dma_start(outp_o[:], outp_sb[:])

    nc.compile()
    return nc


def _get_kernels():
    global _KERNELS
    if _KERNELS is None:
        _KERNELS = (_build_k1(), _build_k2(), _build_k3())
    return _KERNELS


def _part2(v):
    """[256] vector -> [128, 2] partition-major chunks."""
    return np.ascontiguousarray(v.reshape(2, 128).T)


def kernel(input_ids, hidden, encoder_outputs, params, trace=False):
    import ml_dtypes
    _, _, _, _, run_bass_kernel_spmd = _imports()
    bf = ml_dtypes.bfloat16
    k1, k2, k3 = _get_kernels()
    cores = list(range(NC))
    LAST_EXEC_NS.clear()

    def run(nc, in_maps):
        res = run_bass_kernel_spmd(nc, in_maps, cores, trace=trace)
        if trace:
            LAST_EXEC_NS.append(res.exec_time_ns)
        return res.results

    p = {k: np.asarray(v, dtype=np.float32) for k, v in params.items()}
    input_ids = np.asarray(input_ids)
    hidden = np.asarray(hidden, dtype=np.float32)
    enc = np.asarray(encoder_outputs, dtype=np.float32)

    h = hidden.reshape(H)
    hp = _part2(h)
    hf = np.ascontiguousarray(h.reshape(1, H))

    # ---- K1 input prep ----
    def chunks2(mt, dt=bf):  # M_w.T [256,256] -> [128, jc, kc, 128]
        return np.ascontiguousarray(
            mt.reshape(2, 128, 2, 128).transpose(1, 0, 2, 3)).astype(dt)

    ua_a = chunks2(p["Ua_w"].T)
    va_a = chunks2(p["Va_w"].T)
    wa_a = chunks2(p["Wa_w"].T)
    ab_a = _part2(p["Wa_b"] + p["Ua_b"])
    vab_a = _part2(p["Va_b"])

    x = p["emb"][int(input_ids[0])]
    x_pad = np.zeros(NC * VSP, np.float32)
    x_pad[:VOCAB] = x

    ut_pads = {}
    for g, name in zip("zrh", ["Uz_w", "Ur_w", "Uh_w"]):
        tp = np.zeros((NC * VSP, H), bf)
        tp[:VOCAB] = p[name].T.astype(bf)
        ut_pads[g] = tp

    in1 = []
    for c in range(NC):
        enc_c = enc[c * SC:(c + 1) * SC, 0, :]          # [512, 256]
        enc_a = np.ascontiguousarray(
            enc_c.T.reshape(2, 128, SC).transpose(1, 0, 2)).astype(bf)
        m = {
            "enc_in": enc_a, "ua_in": ua_a, "va_in": va_a, "wa_in": wa_a,
            "hp_in": hp.astype(bf), "ab_in": ab_a, "vab_in": vab_a,
            "x_in": np.ascontiguousarray(
                x_pad[c * VSP:(c + 1) * VSP].reshape(NV, 128).T).astype(bf),
        }
        for g, name in zip("zrh", ["uzt_in", "urt_in", "uht_in"]):
            m[name] = np.ascontiguousarray(
                ut_pads[g][c * VSP:(c + 1) * VSP].reshape(NV, 128, H)
                .transpose(1, 0, 2))
        in1.append(m)
    r1 = run(k1, in1)

    # ---- K2 input prep (host: stack/slice only) ----
    ux = np.stack([r1[c]["ux_o"][0] for c in range(NC)])       # [8, 768]
    pzc = np.ascontiguousarray(
        np.concatenate([ux[:, 0:H], ux[:, 2 * H:3 * H]], axis=1))
    pr = np.ascontiguousarray(ux[:, H:2 * H])
    seT = np.stack([r1[c]["se_o"].T.reshape(H) for c in range(NC)])
    a0 = r1[0]["a0_o"]

    # per-shard padded V_w.T: [256, NC, VSP]; pad bias -1e30 so exp(pad)=0
    vt_pad = np.zeros((H, NC, VSP), bf)
    vt_pad[:, :, :VS] = p["V_w"].T.reshape(H, NC, VS).astype(bf)
    vb_pad = np.full((NC, VSP), -1e30, np.float32)
    vb_pad[:, :VS] = p["V_b"].reshape(NC, VS)

    def hchunks(mt):  # M_w.T [256,256] -> [128, 2, 256] bf16
        return np.ascontiguousarray(
            mt.reshape(2, 128, H).transpose(1, 0, 2)).astype(bf)

    common2 = {
        "pzc_in": pzc, "pr_in": pr, "seT_in": seT, "a0_in": a0,
        "enc0_in": _part2(enc[0, 0, :]), "hpb_in": hp.astype(bf),
        "hpf_in": hp, "hf_in": hf,
        "br_in": _part2(p["Ur_b"] + p["Wr_b"] + p["Cr_b"]),
        "bz_in": (p["Uz_b"] + p["Wz_b"] + p["Cz_b"]).reshape(1, H).astype(bf),
        "bc_in": (p["Uh_b"] + p["Wh_b"] + p["Ch_b"]).reshape(1, H).astype(bf),
        "czt_in": hchunks(p["Cz_w"].T), "cht_in": hchunks(p["Ch_w"].T),
        "wzt_in": hchunks(p["Wz_w"].T), "wht_in": hchunks(p["Wh_w"].T),
        "crt_in": chunks2(p["Cr_w"].T), "wrt_in": chunks2(p["Wr_w"].T),
    }
    in2 = []
    for c in range(NC):
        m = dict(common2)
        # vt: [128(pj), 2(jc), NV(i), 128(q)]; lhsT slice [jc,i] gives
        # out[q, 1] contribution for vocab chunk i
        m["vt_in"] = np.ascontiguousarray(
            vt_pad[:, c, :].reshape(2, 128, NV, 128).transpose(1, 0, 2, 3))
        m["vb_in"] = np.ascontiguousarray(
            vb_pad[c].reshape(NV, 128).T)                  # [128, NV]
        in2.append(m)
    r2 = run(k2, in2)

    # ---- K3: global log-softmax normalization ----
    ses = np.array([[r2[c]["vstat_o"][0, 0] for c in range(NC)]], np.float32)
    in3 = [{"logits_in": r2[c]["logits_o"], "ses_in": ses} for c in range(NC)]
    r3 = run(k3, in3)

    LAST_RESULTS.update(r1=r1, r2=r2, r3=r3)
    output = np.concatenate(
        [r3[c]["outp_o"].T.reshape(VSP)[:VS] for c in range(NC)])
    output = output.reshape(1, VOCAB)
    new_hidden = r2[0]["nh_o"].reshape(1, 1, H)
    return output, new_hidden
